# revision 36
# baseline (speedup 1.0000x reference)
"""Graphormer attention head on 8 trn2 NeuronCores (row-parallel), v10.

out = softmax(mask(q@k.T/8, adj)) @ v with q/k/v = x@W+b, adj scattered
from edge_index.

Core c owns output rows [c*1024, (c+1)*1024). All-fp16 single-term score
matmuls, row-tiled in pairs across PE row-groups 0-63/64-127 (K=64
contraction -> 2 concurrent matmuls); K^T/Q^T duplicated onto both
partition halves via duplicated weight columns.

The whole kernel is one software-pipelined stream: projection segments
(K via 3 rotating PSUM slots, V via the acc banks) interleave with the
attention tile pairs two segments behind, so ScalarE runs exp back to
back from ~5us on. exp is one solo call per tile on the 3-slot rotation:
scores(t+2) write a slot that no live ACT is reading (t+2 != t mod 3),
which removes the ACT->PE slot-handoff stall of batched calls. The
host-built {0,1} fp16 mask multiplies exp output on DVE (2x_1P mode);
attn@[v|1] accumulates numerator+denominator in PSUM a few tiles behind.
All PSUM->SBUF copies ride DVE. Biases are zeros per the problem spec
(asserted on host); bv is folded exactly via the final I65 matmul.
"""
import os
import sys

for _p in ("/opt/trn_rl_repo", "/root/.axon_site/_ro/trn_rl_repo"):
    if os.path.isdir(_p) and _p not in sys.path:
        sys.path.insert(0, _p)

import numpy as np
import ml_dtypes

import concourse.bass as bass
import concourse.bacc as bacc
import concourse.mybir as mybir
import concourse.tile as tile
from concourse.bass_utils import run_bass_kernel_spmd

N = 8192
DIN = 256
DQ = 64
NCORES = 8
NLOC = N // NCORES          # 1024 rows per core
JT = N // 128               # 64 column tiles of 128
F32 = mybir.dt.float32
F16 = mybir.dt.float16
WV_DEPTH = 3                # attn@v runs this many tiles behind exp


def _emit(nc, tc, ctx):
    from concourse.mybir import AluOpType as AO, ActivationFunctionType as AF

    xt = nc.dram_tensor("xt", [DIN, N], F16, kind="ExternalInput")
    xtq = nc.dram_tensor("xtq", [DIN, NLOC], F16, kind="ExternalInput")
    wqd = nc.dram_tensor("wqd", [DIN, 128], F16, kind="ExternalInput")
    wkd = nc.dram_tensor("wkd", [DIN, 128], F16, kind="ExternalInput")
    wv = nc.dram_tensor("wv", [DIN, DQ], F16, kind="ExternalInput")
    i65 = nc.dram_tensor("i65", [DQ + 1, DQ + 1], F16, kind="ExternalInput")
    maskt = nc.dram_tensor("maskt", [N, NLOC], F16, kind="ExternalInput")
    out = nc.dram_tensor("out", [NLOC, DQ], F32, kind="ExternalOutput")

    pers = ctx.enter_context(tc.tile_pool(name="pers", bufs=1))
    pm = ctx.enter_context(tc.tile_pool(name="pm", bufs=8))
    pe_ = ctx.enter_context(tc.tile_pool(name="pe", bufs=6))
    pw = ctx.enter_context(tc.tile_pool(name="pw", bufs=17))
    pfin = ctx.enter_context(tc.tile_pool(name="pfin", bufs=2))
    psB = ctx.enter_context(tc.tile_pool(name="psB", bufs=1, space="PSUM"))
    pacc = ctx.enter_context(tc.tile_pool(name="pacc", bufs=1, space="PSUM"))

    # ---- persistent SBUF ----
    xt_sb = [pers.tile([128, N], F16, tag=f"xt{c}", name=f"xt{c}") for c in range(2)]
    xtq_sb = [pers.tile([128, NLOC], F16, tag=f"xtq{c}", name=f"xtq{c}")
              for c in range(2)]
    wqd_sb = [pers.tile([128, 128], F16, tag=f"wqd{c}", name=f"wqd{c}")
              for c in range(2)]
    wkd_sb = [pers.tile([128, 128], F16, tag=f"wkd{c}", name=f"wkd{c}")
              for c in range(2)]
    wv_sb = [pers.tile([128, DQ], F16, tag=f"wv{c}", name=f"wv{c}")
             for c in range(2)]
    i65_sb = pers.tile([DQ + 1, DQ + 1], F16, tag="i65")
    kth_sb = pers.tile([128, N], F16, tag="kth")        # K^T duplicated halves
    qth_sb = pers.tile([128, NLOC], F16, tag="qth")     # Q^T duplicated halves
    vh_sb = pers.tile([128, JT * (DQ + 1)], F16, tag="vh")
    accT_sb = pers.tile([DQ + 1, NLOC], F16, tag="accT")

    # ---- input DMAs: weights + xtq first, x^T chunks segment-major on
    # both HWDGE queues ----
    for c in range(2):
        nc.sync.dma_start(wkd_sb[c][:], wkd[c * 128:(c + 1) * 128, :])
        nc.sync.dma_start(wqd_sb[c][:], wqd[c * 128:(c + 1) * 128, :])
        nc.sync.dma_start(wv_sb[c][:], wv[c * 128:(c + 1) * 128, :])
        nc.scalar.dma_start(xtq_sb[c][:], xtq[c * 128:(c + 1) * 128, :])
    nc.sync.dma_start(i65_sb[:], i65[:])
    for s in range(N // NLOC):
        for c in range(2):
            eng = nc.scalar if (2 * s + c) % 2 else nc.sync
            eng.dma_start(
                xt_sb[c][:, s * NLOC:(s + 1) * NLOC],
                xt[c * 128:(c + 1) * 128, s * NLOC:(s + 1) * NLOC],
            )

    # 3 rotating PSUM score/projection slots (6 banks) + acc banks that
    # double as V-projection scratch before the first attn@v matmul
    sbig = psB.tile([128, 3 * NLOC], F32, tag="sbig")
    accbig = pacc.tile([128, NLOC], F32, tag="acc")
    acc = accbig[0:DQ + 1, :]
    slot_ctr = [0]

    def next_slot():
        sl = slot_ctr[0] % 3
        slot_ctr[0] += 1
        return sbig[:, sl * NLOC:(sl + 1) * NLOC]

    vh3 = vh_sb[:].rearrange("p (b e) -> p b e", e=DQ + 1)
    nc.vector.memset(vh3[:, :, DQ:DQ + 1], 1.0)
    mt4 = maskt.rearrange("(q t p) c -> q p t c", t=2, p=128)

    def emit_q():
        qp = next_slot()
        for b in range(2):
            o = qp[:, b * 512:(b + 1) * 512]
            nc.tensor.matmul(o, wqd_sb[0][:],
                             xtq_sb[0][:, b * 512:(b + 1) * 512],
                             start=True, stop=False)
            nc.tensor.matmul(o, wqd_sb[1][:],
                             xtq_sb[1][:, b * 512:(b + 1) * 512],
                             start=False, stop=True)
        nc.vector.tensor_copy(qth_sb[:], qp)

    def emit_kseg(s):
        kp = next_slot()
        for b in range(2):
            o = kp[:, b * 512:(b + 1) * 512]
            cols = slice(s * NLOC + b * 512, s * NLOC + (b + 1) * 512)
            nc.tensor.matmul(o, wkd_sb[0][:], xt_sb[0][:, cols],
                             start=True, stop=False)
            nc.tensor.matmul(o, wkd_sb[1][:], xt_sb[1][:, cols],
                             start=False, stop=True)
        nc.vector.tensor_copy(kth_sb[:, s * NLOC:(s + 1) * NLOC], kp)

    def emit_vseg(s):
        vp = accbig[:, (s % 2) * 512:(s % 2 + 1) * 512]
        for b in range(8):
            jt = s * 8 + b
            o = vp[:, b * DQ:(b + 1) * DQ]
            nc.tensor.matmul(o, xt_sb[0][:, jt * 128:(jt + 1) * 128],
                             wv_sb[0][:], start=True, stop=False)
            nc.tensor.matmul(o, xt_sb[1][:, jt * 128:(jt + 1) * 128],
                             wv_sb[1][:], start=False, stop=True)
        nc.vector.tensor_copy(vh3[:, s * 8:(s + 1) * 8, 0:DQ], vp)

    pending = []

    def emit_wv(w_t, jt):
        vhb = vh3[:, jt, :]
        for b in range(2):
            nc.tensor.matmul(acc[:, b * 512:(b + 1) * 512], vhb,
                             w_t[:, b * 512:(b + 1) * 512],
                             start=(jt == 0), stop=(jt == JT - 1))


    def emit_pair(p):
        jta, jtb = 2 * p, 2 * p + 1
        m2 = pm.tile([128, 2 * NLOC], F16, tag="m", name="m2")
        m2v = m2[:].rearrange("p (t c) -> p t c", t=2)
        nc.sync.dma_start(m2v, mt4[p])
        sl_a = slot_ctr[0] % 3
        sa, sb = next_slot(), next_slot()
        kh_a = kth_sb[0:64, jta * 128:(jta + 1) * 128]
        kh_b = kth_sb[64:128, jtb * 128:(jtb + 1) * 128]
        for b in range(2):
            hs = slice(b * 512, (b + 1) * 512)
            nc.tensor.matmul(sa[:, hs], kh_a, qth_sb[0:64, hs],
                             start=True, stop=True)
            nc.tensor.matmul(sb[:, hs], kh_b, qth_sb[64:128, hs],
                             start=True, stop=True)
        d2 = pe_.tile([128, 2 * NLOC], F16, tag="d", name="d2")
        if sl_a < 2:
            nc.scalar.activation(
                d2[:], sbig[:, sl_a * NLOC:(sl_a + 2) * NLOC], AF.Exp)
        else:
            nc.scalar.activation(d2[:, 0:NLOC], sa, AF.Exp)
            nc.scalar.activation(d2[:, NLOC:2 * NLOC], sb, AF.Exp)
        w2 = pw.tile([128, 2 * NLOC], F16, tag="w", name="w2")
        for t in range(2):
            ts = slice(t * NLOC, (t + 1) * NLOC)
            nc.vector.tensor_tensor(w2[:, ts], d2[:, ts], m2[:, ts], AO.mult)
        pending.append((w2[:, 0:NLOC], jta))
        pending.append((w2[:, NLOC:2 * NLOC], jtb))
        depth = 32 if p < 16 else max(3, 32 - 2 * (p - 15))
        while len(pending) > depth:
            emit_wv(*pending.pop(0))

    # ---- single interleaved stream: K/V units with the first 16 pairs
    # (attn@v deferred past V7 so the acc-bank scratch stays safe), then
    # the remaining pairs while the wv backlog drains ----
    emit_q()
    for s in range(8):
        emit_kseg(s)
        emit_vseg(s)
        emit_pair(2 * s)
        emit_pair(2 * s + 1)
    for p in range(16, JT // 2):
        emit_pair(p)
    for args in pending:
        emit_wv(*args)

    # ---- finish: transpose via matmul with I65 (adds bv*Z), divide by Z ----
    nc.scalar.activation(accT_sb[:], acc[:], AF.Copy)
    ofin = pfin.tile([128, 8 * DQ], F32, tag="o")
    for it in range(NLOC // 128):
        po = sbig[:, it * 128:it * 128 + DQ + 1]
        nc.tensor.matmul(po, accT_sb[:, it * 128:(it + 1) * 128], i65_sb[:],
                         start=True, stop=True)
    for it in range(NLOC // 128):
        po = sbig[:, it * 128:it * 128 + DQ + 1]
        rz = pfin.tile([128, 1], F32, tag="rz")
        nc.vector.reciprocal(rz[:], po[:, DQ:DQ + 1])
        nc.vector.tensor_scalar_mul(ofin[:, it * DQ:(it + 1) * DQ],
                                    po[:, 0:DQ], rz[:])
    ofin3 = ofin[:].rearrange("p (g d) -> p g d", d=DQ)
    nc.sync.dma_start(out.rearrange("(g p) d -> p g d", p=128), ofin3)


_CACHE = {}


def _program():
    if "nc" not in _CACHE:
        import contextlib
        nc = bacc.Bacc("TRN2", target_bir_lowering=False, debug=False,
                       num_devices=NCORES)
        with tile.TileContext(nc) as tc:
            with contextlib.ExitStack() as ctx:
                _emit(nc, tc, ctx)
        nc.compile()
        _CACHE["nc"] = nc
    return _CACHE["nc"]


def kernel(**inputs):
    x = np.asarray(inputs["x"], dtype=np.float32)
    ei = np.asarray(inputs["edge_index"])
    Wq = np.asarray(inputs["Wq"], dtype=np.float32)
    bq = np.asarray(inputs["bq"], dtype=np.float32)
    Wk = np.asarray(inputs["Wk"], dtype=np.float32)
    bk = np.asarray(inputs["bk"], dtype=np.float32)
    Wv = np.asarray(inputs["Wv"], dtype=np.float32)
    bv = np.asarray(inputs["bv"], dtype=np.float32)

    # q/k biases are zeros by the problem spec (fill: zeros); the kernel
    # relies on that (bv is handled exactly via the i65 transpose).
    assert not np.any(bq) and not np.any(bk), "nonzero q/k bias unsupported"

    scale = 1.0 / np.sqrt(np.float32(DQ))
    xT = np.ascontiguousarray(x.T).astype(np.float16)        # (256, 8192)
    wq_s = (Wq * scale).astype(np.float16)
    wqd = np.ascontiguousarray(np.concatenate([wq_s, wq_s], axis=1))
    wk16 = Wk.astype(np.float16)
    wkd = np.ascontiguousarray(np.concatenate([wk16, wk16], axis=1))
    wv16 = np.ascontiguousarray(Wv.astype(np.float16))
    i65 = np.eye(DQ + 1, dtype=np.float32)
    i65[DQ, :DQ] = bv
    i65 = i65.astype(np.float16)
    adj = np.zeros((N, N), dtype=np.bool_)
    adj[ei[0], ei[1]] = True

    in_maps = []
    for c in range(NCORES):
        rows = slice(c * NLOC, (c + 1) * NLOC)
        in_maps.append({
            "xt": xT,
            "xtq": np.ascontiguousarray(xT[:, rows]),
            "wqd": wqd, "wkd": wkd, "wv": wv16,
            "i65": i65,
            "maskt": np.ascontiguousarray(adj[rows].T).astype(np.float16),
        })

    global _last_in_maps
    _last_in_maps = in_maps
    nc = _program()
    res = run_bass_kernel_spmd(nc, in_maps, core_ids=list(range(NCORES)))
    out = np.concatenate([res.results[c]["out"] for c in range(NCORES)], axis=0)
    return out.astype(np.float32)


_last_in_maps = None


# revision 40
# speedup vs baseline: 1.0580x; 1.0580x over previous
"""Graphormer attention head on 8 trn2 NeuronCores (row-parallel), v10.

out = softmax(mask(q@k.T/8, adj)) @ v with q/k/v = x@W+b, adj scattered
from edge_index.

Core c owns output rows [c*1024, (c+1)*1024). All-fp16 single-term score
matmuls, row-tiled in pairs across PE row-groups 0-63/64-127 (K=64
contraction -> 2 concurrent matmuls); K^T/Q^T duplicated onto both
partition halves via duplicated weight columns.

The whole kernel is one software-pipelined stream: projection segments
(K via 3 rotating PSUM slots, V via the acc banks) interleave with the
attention tile pairs two segments behind, so ScalarE runs exp back to
back from ~5us on. exp is one solo call per tile on the 3-slot rotation:
scores(t+2) write a slot that no live ACT is reading (t+2 != t mod 3),
which removes the ACT->PE slot-handoff stall of batched calls. The
host-built {0,1} fp16 mask multiplies exp output on DVE (2x_1P mode);
attn@[v|1] accumulates numerator+denominator in PSUM a few tiles behind.
All PSUM->SBUF copies ride DVE. Biases are zeros per the problem spec
(asserted on host); bv is folded exactly via the final I65 matmul.
"""
import os
import sys

for _p in ("/opt/trn_rl_repo", "/root/.axon_site/_ro/trn_rl_repo"):
    if os.path.isdir(_p) and _p not in sys.path:
        sys.path.insert(0, _p)

import numpy as np
import ml_dtypes

import concourse.bass as bass
import concourse.bacc as bacc
import concourse.mybir as mybir
import concourse.tile as tile
from concourse.bass_utils import run_bass_kernel_spmd

N = 8192
DIN = 256
DQ = 64
NCORES = 8
NLOC = N // NCORES          # 1024 rows per core
JT = N // 128               # 64 column tiles of 128
F32 = mybir.dt.float32
F16 = mybir.dt.float16
WV_DEPTH = 3                # attn@v runs this many tiles behind exp


def _emit(nc, tc, ctx):
    from concourse.mybir import AluOpType as AO, ActivationFunctionType as AF

    xt = nc.dram_tensor("xt", [DIN, N], F16, kind="ExternalInput")
    xtq = nc.dram_tensor("xtq", [DIN, NLOC], F16, kind="ExternalInput")
    wqd = nc.dram_tensor("wqd", [DIN, 128], F16, kind="ExternalInput")
    wkd = nc.dram_tensor("wkd", [DIN, 128], F16, kind="ExternalInput")
    wv = nc.dram_tensor("wv", [DIN, DQ], F16, kind="ExternalInput")
    i65 = nc.dram_tensor("i65", [DQ + 1, DQ + 1], F16, kind="ExternalInput")
    maskt = nc.dram_tensor("maskt", [N, NLOC], F16, kind="ExternalInput")
    out = nc.dram_tensor("out", [NLOC, DQ], F32, kind="ExternalOutput")

    pers = ctx.enter_context(tc.tile_pool(name="pers", bufs=1))
    pm = ctx.enter_context(tc.tile_pool(name="pm", bufs=8))
    pe_ = ctx.enter_context(tc.tile_pool(name="pe", bufs=6))
    pw = ctx.enter_context(tc.tile_pool(name="pw", bufs=6))
    pfin = ctx.enter_context(tc.tile_pool(name="pfin", bufs=2))
    psB = ctx.enter_context(tc.tile_pool(name="psB", bufs=1, space="PSUM"))
    pacc = ctx.enter_context(tc.tile_pool(name="pacc", bufs=1, space="PSUM"))

    # ---- persistent SBUF ----
    xt_sb = [pers.tile([128, N], F16, tag=f"xt{c}", name=f"xt{c}") for c in range(2)]
    xtq_sb = [pers.tile([128, NLOC], F16, tag=f"xtq{c}", name=f"xtq{c}")
              for c in range(2)]
    wqd_sb = [pers.tile([128, 128], F16, tag=f"wqd{c}", name=f"wqd{c}")
              for c in range(2)]
    wkd_sb = [pers.tile([128, 128], F16, tag=f"wkd{c}", name=f"wkd{c}")
              for c in range(2)]
    wv_sb = [pers.tile([128, DQ], F16, tag=f"wv{c}", name=f"wv{c}")
             for c in range(2)]
    i65_sb = pers.tile([DQ + 1, DQ + 1], F16, tag="i65")
    kth_sb = pers.tile([128, N], F16, tag="kth")        # K^T duplicated halves
    qth_sb = pers.tile([128, NLOC], F16, tag="qth")     # Q^T duplicated halves
    vh_sb = pers.tile([128, JT * (DQ + 1)], F16, tag="vh")
    accT_sb = pers.tile([DQ + 1, NLOC], F16, tag="accT")

    # ---- input DMAs: weights + xtq first, x^T chunks segment-major on
    # both HWDGE queues ----
    for c in range(2):
        nc.sync.dma_start(wkd_sb[c][:], wkd[c * 128:(c + 1) * 128, :])
        nc.sync.dma_start(wqd_sb[c][:], wqd[c * 128:(c + 1) * 128, :])
        nc.sync.dma_start(wv_sb[c][:], wv[c * 128:(c + 1) * 128, :])
        nc.scalar.dma_start(xtq_sb[c][:], xtq[c * 128:(c + 1) * 128, :])
    nc.sync.dma_start(i65_sb[:], i65[:])
    for s in range(N // NLOC):
        for c in range(2):
            eng = nc.scalar if (2 * s + c) % 2 else nc.sync
            eng.dma_start(
                xt_sb[c][:, s * NLOC:(s + 1) * NLOC],
                xt[c * 128:(c + 1) * 128, s * NLOC:(s + 1) * NLOC],
            )

    # 3 rotating PSUM score/projection slots (6 banks) + acc banks that
    # double as V-projection scratch before the first attn@v matmul
    sbig = psB.tile([128, 3 * NLOC], F32, tag="sbig")
    accbig = pacc.tile([128, NLOC], F32, tag="acc")
    acc = accbig[0:DQ + 1, :]
    slot_ctr = [0]

    def next_slot():
        sl = slot_ctr[0] % 3
        slot_ctr[0] += 1
        return sbig[:, sl * NLOC:(sl + 1) * NLOC]

    vh3 = vh_sb[:].rearrange("p (b e) -> p b e", e=DQ + 1)
    nc.vector.memset(vh3[:, :, DQ:DQ + 1], 1.0)
    mt4 = maskt.rearrange("(q t p) c -> q p t c", t=2, p=128)

    def emit_q():
        qp = next_slot()
        for b in range(2):
            o = qp[:, b * 512:(b + 1) * 512]
            nc.tensor.matmul(o, wqd_sb[0][:],
                             xtq_sb[0][:, b * 512:(b + 1) * 512],
                             start=True, stop=False)
            nc.tensor.matmul(o, wqd_sb[1][:],
                             xtq_sb[1][:, b * 512:(b + 1) * 512],
                             start=False, stop=True)
        nc.vector.tensor_copy(qth_sb[:], qp)

    def emit_kseg(s):
        kp = next_slot()
        for b in range(2):
            o = kp[:, b * 512:(b + 1) * 512]
            cols = slice(s * NLOC + b * 512, s * NLOC + (b + 1) * 512)
            nc.tensor.matmul(o, wkd_sb[0][:], xt_sb[0][:, cols],
                             start=True, stop=False)
            nc.tensor.matmul(o, wkd_sb[1][:], xt_sb[1][:, cols],
                             start=False, stop=True)
        nc.scalar.activation(kth_sb[:, s * NLOC:(s + 1) * NLOC], kp, AF.Copy)

    def emit_vseg(s):
        vp = accbig[:, (s % 2) * 512:(s % 2 + 1) * 512]
        for b in range(8):
            jt = s * 8 + b
            o = vp[:, b * DQ:(b + 1) * DQ]
            nc.tensor.matmul(o, xt_sb[0][:, jt * 128:(jt + 1) * 128],
                             wv_sb[0][:], start=True, stop=False)
            nc.tensor.matmul(o, xt_sb[1][:, jt * 128:(jt + 1) * 128],
                             wv_sb[1][:], start=False, stop=True)
        nc.scalar.activation(vh3[:, s * 8:(s + 1) * 8, 0:DQ], vp, AF.Copy)

    pending = []

    def emit_wv(w_t, jt):
        vhb = vh3[:, jt, :]
        for b in range(2):
            nc.tensor.matmul(acc[:, b * 512:(b + 1) * 512], vhb,
                             w_t[:, b * 512:(b + 1) * 512],
                             start=(jt == 0), stop=(jt == JT - 1))


    def emit_pair(p):
        jta, jtb = 2 * p, 2 * p + 1
        m2 = pm.tile([128, 2 * NLOC], F16, tag="m", name="m2")
        m2v = m2[:].rearrange("p (t c) -> p t c", t=2)
        nc.sync.dma_start(m2v, mt4[p])
        sl_a = slot_ctr[0] % 3
        sa, sb = next_slot(), next_slot()
        kh_a = kth_sb[0:64, jta * 128:(jta + 1) * 128]
        kh_b = kth_sb[64:128, jtb * 128:(jtb + 1) * 128]
        for b in range(2):
            hs = slice(b * 512, (b + 1) * 512)
            nc.tensor.matmul(sa[:, hs], kh_a, qth_sb[0:64, hs],
                             start=True, stop=True)
            nc.tensor.matmul(sb[:, hs], kh_b, qth_sb[64:128, hs],
                             start=True, stop=True)
        d2 = pe_.tile([128, 2 * NLOC], F16, tag="d", name="d2")
        if sl_a < 2:
            nc.scalar.activation(
                d2[:], sbig[:, sl_a * NLOC:(sl_a + 2) * NLOC], AF.Exp)
        else:
            nc.scalar.activation(d2[:, 0:NLOC], sa, AF.Exp)
            nc.scalar.activation(d2[:, NLOC:2 * NLOC], sb, AF.Exp)
        w2 = pw.tile([128, 2 * NLOC], F16, tag="w", name="w2")
        for t in range(2):
            ts = slice(t * NLOC, (t + 1) * NLOC)
            nc.vector.tensor_tensor(w2[:, ts], d2[:, ts], m2[:, ts], AO.mult)
        pending.append((w2[:, 0:NLOC], jta))
        pending.append((w2[:, NLOC:2 * NLOC], jtb))
        while len(pending) > WV_DEPTH:
            emit_wv(*pending.pop(0))

    # ---- prologue (projections), then the main loop ----
    emit_q()
    for s in range(8):
        emit_kseg(s)
        emit_vseg(s)
    for p in range(JT // 2):
        emit_pair(p)
    for args in pending:
        emit_wv(*args)

    # ---- finish: transpose via matmul with I65 (adds bv*Z), divide by Z ----
    nc.scalar.activation(accT_sb[:], acc[:], AF.Copy)
    ofin = pfin.tile([128, 8 * DQ], F32, tag="o")
    for it in range(NLOC // 128):
        po = sbig[:, it * 128:it * 128 + DQ + 1]
        nc.tensor.matmul(po, accT_sb[:, it * 128:(it + 1) * 128], i65_sb[:],
                         start=True, stop=True)
    for it in range(NLOC // 128):
        po = sbig[:, it * 128:it * 128 + DQ + 1]
        rz = pfin.tile([128, 1], F32, tag="rz")
        nc.vector.reciprocal(rz[:], po[:, DQ:DQ + 1])
        nc.vector.tensor_scalar_mul(ofin[:, it * DQ:(it + 1) * DQ],
                                    po[:, 0:DQ], rz[:])
    ofin3 = ofin[:].rearrange("p (g d) -> p g d", d=DQ)
    nc.sync.dma_start(out.rearrange("(g p) d -> p g d", p=128), ofin3)


_CACHE = {}


def _program():
    if "nc" not in _CACHE:
        import contextlib
        nc = bacc.Bacc("TRN2", target_bir_lowering=False, debug=False,
                       num_devices=NCORES)
        with tile.TileContext(nc) as tc:
            with contextlib.ExitStack() as ctx:
                _emit(nc, tc, ctx)
        nc.compile()
        _CACHE["nc"] = nc
    return _CACHE["nc"]


def kernel(**inputs):
    x = np.asarray(inputs["x"], dtype=np.float32)
    ei = np.asarray(inputs["edge_index"])
    Wq = np.asarray(inputs["Wq"], dtype=np.float32)
    bq = np.asarray(inputs["bq"], dtype=np.float32)
    Wk = np.asarray(inputs["Wk"], dtype=np.float32)
    bk = np.asarray(inputs["bk"], dtype=np.float32)
    Wv = np.asarray(inputs["Wv"], dtype=np.float32)
    bv = np.asarray(inputs["bv"], dtype=np.float32)

    # q/k biases are zeros by the problem spec (fill: zeros); the kernel
    # relies on that (bv is handled exactly via the i65 transpose).
    assert not np.any(bq) and not np.any(bk), "nonzero q/k bias unsupported"

    scale = 1.0 / np.sqrt(np.float32(DQ))
    xT = np.ascontiguousarray(x.T).astype(np.float16)        # (256, 8192)
    wq_s = (Wq * scale).astype(np.float16)
    wqd = np.ascontiguousarray(np.concatenate([wq_s, wq_s], axis=1))
    wk16 = Wk.astype(np.float16)
    wkd = np.ascontiguousarray(np.concatenate([wk16, wk16], axis=1))
    wv16 = np.ascontiguousarray(Wv.astype(np.float16))
    i65 = np.eye(DQ + 1, dtype=np.float32)
    i65[DQ, :DQ] = bv
    i65 = i65.astype(np.float16)
    adj = np.zeros((N, N), dtype=np.bool_)
    adj[ei[0], ei[1]] = True

    in_maps = []
    for c in range(NCORES):
        rows = slice(c * NLOC, (c + 1) * NLOC)
        in_maps.append({
            "xt": xT,
            "xtq": np.ascontiguousarray(xT[:, rows]),
            "wqd": wqd, "wkd": wkd, "wv": wv16,
            "i65": i65,
            "maskt": np.ascontiguousarray(adj[rows].T).astype(np.float16),
        })

    global _last_in_maps
    _last_in_maps = in_maps
    nc = _program()
    res = run_bass_kernel_spmd(nc, in_maps, core_ids=list(range(NCORES)))
    out = np.concatenate([res.results[c]["out"] for c in range(NCORES)], axis=0)
    return out.astype(np.float32)


_last_in_maps = None
